# revision 9
# baseline (speedup 1.0000x reference)
"""GraphSAGE layer on 8 Trainium2 NeuronCores.

Strategy (edge sharding by DESTINATION node range — no collectives needed):
  - 50000 nodes -> 392 tiles of 128 nodes; core k owns 49 consecutive tiles.
  - Host bins edges by dst tile, pads each tile's edge list to B_max blocks
    of 128 edges (uniform compile-time structure across cores).
  - Device, per node tile:
      * indirect-DMA gather of msg = nh[src] for the tile's edges
        -> [128 edge, 128 feat] blocks (edge on partition).
      * one-hot segment matmul: psum[feat, node] += msg_b.T @ onehot_b where
        onehot[e, n] = (dst_local[e]==n); all b_max one-hots are built in ONE
        batched DVE tensor_tensor (iota vs broadcast dst compare).
      * mean = psum * inv_count (host-replicated [128, nodes] table).
      * MLP layer 1 in transposed layout (lhsT = W1 chunks, rhs = nh_T/agg_T),
        relu+bias on ScalarE; layer 2 flipped (lhsT = h1) so the output psum
        is [node, feat]; bias b2 added via a K=1 ones-outer-product matmul.
  - eh is returned untouched (reference passes it through).
"""

import json

import numpy as np

N_NODES = 50000
N_EDGES = 600000
D_IN = 128
D_HID = 256
D_OUT = 128
P = 128
N_CORES = 8
N_TILES_TOTAL = 392  # 8 * 49 >= ceil(50000/128) = 391
TILES_PER_CORE = N_TILES_TOTAL // N_CORES  # 49

# fp32 = exactness; fp16 halves gather traffic + speeds PE (FWL).
GATHER_DT = "float32"


def _split_waits(bj):
    """Walrus in this toolchain accepts at most ONE sync wait per instruction
    (bass_rust: 'everything else is capped at 1'), but the Tile scheduler
    attaches all needed waits to the consumer. Hoist excess waits onto NoOp
    carrier instructions inserted just before, on the same engine queue."""
    d = json.loads(bj)
    ctr = 0
    for f in d["functions"]:
        for blk in f["blocks"]:
            out = []
            for inst in blk["instructions"]:
                si = inst.get("sync_info")
                waits = (si or {}).get("on_wait") or []
                if len(waits) > 1:
                    for w in waits[:-1]:
                        ctr += 1
                        out.append(
                            {
                                "debug": inst.get("debug", 0),
                                "engine": inst["engine"],
                                "ins": [],
                                "outs": [],
                                "name": f"Wcar-{ctr}",
                                "opcode": "NoOp",
                                "text_hint": "wait_carrier",
                                "sync_info": {"on_update": [], "on_wait": [w]},
                            }
                        )
                    si["on_wait"] = waits[-1:]
                out.append(inst)
            blk["instructions"] = out
    return json.dumps(d).encode()


def _host_prep(nh, edge_index):
    """Build per-core edge tables. Returns (b_max, ncols, per-core arrays)."""
    src = np.asarray(edge_index[0], dtype=np.int64)
    dst = np.asarray(edge_index[1], dtype=np.int64)

    counts = np.bincount(dst, minlength=N_NODES)
    invc = (1.0 / np.maximum(counts, 1)).astype(np.float32)
    invc_pad_nodes = np.ones(N_TILES_TOTAL * P, dtype=np.float32)
    invc_pad_nodes[:N_NODES] = invc

    tile_of_edge = dst >> 7  # dst // 128
    order = np.argsort(tile_of_edge, kind="stable")
    s_src = src[order]
    s_dst = dst[order]

    tile_cnt = np.bincount(tile_of_edge, minlength=N_TILES_TOTAL)
    b_max = int(np.max((tile_cnt + P - 1) // P))

    slots_per_tile = b_max * P
    ncols = TILES_PER_CORE * b_max  # columns of the per-core [128, ncols] tables

    tot_slots = N_TILES_TOTAL * slots_per_tile
    src_pad = np.zeros(tot_slots, dtype=np.int32)
    dstl_pad = np.full(tot_slots, -1.0, dtype=np.float32)  # -1 -> one-hot row = 0

    tile_starts = np.zeros(N_TILES_TOTAL + 1, dtype=np.int64)
    np.cumsum(tile_cnt, out=tile_starts[1:])
    pos_in_tile = np.arange(N_EDGES) - tile_starts[tile_of_edge[order]]
    slot = tile_of_edge[order] * slots_per_tile + pos_in_tile
    src_pad[slot] = s_src.astype(np.int32)
    dstl_pad[slot] = (s_dst & 127).astype(np.float32)

    src_tbp = src_pad.reshape(N_TILES_TOTAL, b_max, P)
    dstl_tbp = dstl_pad.reshape(N_TILES_TOTAL, b_max, P)

    nhT_full = np.zeros((P, N_TILES_TOTAL * P), dtype=np.float32)
    nhT_full[:, :N_NODES] = np.ascontiguousarray(nh.T)

    per_core = []
    for k in range(N_CORES):
        t0, t1 = k * TILES_PER_CORE, (k + 1) * TILES_PER_CORE
        src_k = np.ascontiguousarray(
            src_tbp[t0:t1].reshape(TILES_PER_CORE * b_max, P).T
        )
        dstl_k = np.ascontiguousarray(
            dstl_tbp[t0:t1].reshape(TILES_PER_CORE * b_max, P).T
        )
        nhT_k = np.ascontiguousarray(nhT_full[:, t0 * P : t1 * P])
        invcr_k = np.ascontiguousarray(
            np.broadcast_to(
                invc_pad_nodes[t0 * P : t1 * P][None, :], (P, TILES_PER_CORE * P)
            )
        )
        per_core.append(
            {"srcidx": src_k, "dstsel": dstl_k, "nhT": nhT_k, "invcr": invcr_k}
        )
    return b_max, ncols, per_core


def _build_bass(b_max, ncols, gather_np_dt):
    import concourse.bass as bass
    import concourse.mybir as mybir
    from concourse.tile import TileContext

    f32 = mybir.dt.float32
    gdt = {"float32": mybir.dt.float32, "float16": mybir.dt.float16}[gather_np_dt]

    nc = bass.Bass()
    nh_full = nc.dram_tensor("nh_full", [N_NODES, D_IN], gdt, kind="ExternalInput")
    nhT = nc.dram_tensor("nhT", [P, TILES_PER_CORE * P], f32, kind="ExternalInput")
    srcidx = nc.dram_tensor("srcidx", [P, ncols], mybir.dt.int32, kind="ExternalInput")
    dstsel = nc.dram_tensor("dstsel", [P, ncols], f32, kind="ExternalInput")
    invcr = nc.dram_tensor(
        "invcr", [P, TILES_PER_CORE * P], f32, kind="ExternalInput"
    )
    W1 = nc.dram_tensor("W1", [2 * D_IN, D_HID], f32, kind="ExternalInput")
    b1 = nc.dram_tensor("b1", [D_HID, 1], f32, kind="ExternalInput")
    W2 = nc.dram_tensor("W2", [D_HID, D_OUT], f32, kind="ExternalInput")
    b2 = nc.dram_tensor("b2", [1, D_OUT], f32, kind="ExternalInput")
    nh_out = nc.dram_tensor(
        "nh_out", [TILES_PER_CORE * P, D_OUT], f32, kind="ExternalOutput"
    )

    with TileContext(nc) as tc:
        with (
            tc.tile_pool(name="const", bufs=1) as cpool,
            tc.tile_pool(name="msg", bufs=3) as msgpool,
            tc.tile_pool(name="oh", bufs=3) as ohpool,
            tc.tile_pool(name="work", bufs=3) as wpool,
            tc.tile_pool(name="outp", bufs=3) as opool,
            tc.tile_pool(name="ps_agg", bufs=2, space="PSUM") as ps_agg,
            tc.tile_pool(name="ps_h1", bufs=2, space="PSUM") as ps_h1,
            tc.tile_pool(name="ps_h2", bufs=2, space="PSUM") as ps_h2,
        ):
            # ---- resident constants ----
            srcidx_sb = cpool.tile([P, ncols], mybir.dt.int32)
            nc.sync.dma_start(out=srcidx_sb[:], in_=srcidx[:, :])
            dstsel_sb = cpool.tile([P, ncols], f32)
            nc.sync.dma_start(out=dstsel_sb[:], in_=dstsel[:, :])
            invcr_sb = cpool.tile([P, TILES_PER_CORE * P], f32)
            nc.sync.dma_start(out=invcr_sb[:], in_=invcr[:, :])
            nhT_sb = cpool.tile([P, TILES_PER_CORE * P], f32)
            nc.sync.dma_start(out=nhT_sb[:], in_=nhT[:, :])
            W1_sb0 = cpool.tile([P, D_HID], f32)
            nc.sync.dma_start(out=W1_sb0[:], in_=W1[0:P, :])
            W1_sb1 = cpool.tile([P, D_HID], f32)
            nc.sync.dma_start(out=W1_sb1[:], in_=W1[P : 2 * P, :])
            W2_sb0 = cpool.tile([P, D_OUT], f32)
            nc.sync.dma_start(out=W2_sb0[:], in_=W2[0:P, :])
            W2_sb1 = cpool.tile([P, D_OUT], f32)
            nc.sync.dma_start(out=W2_sb1[:], in_=W2[P : 2 * P, :])
            b1_sb = cpool.tile([P, 2], f32)
            nc.sync.dma_start(out=b1_sb[:, 0:1], in_=b1[0:P, :])
            nc.sync.dma_start(out=b1_sb[:, 1:2], in_=b1[P : 2 * P, :])
            b2_sb = cpool.tile([1, D_OUT], f32)
            nc.sync.dma_start(out=b2_sb[:], in_=b2[:, :])
            ones_sb = cpool.tile([1, P], f32)
            nc.vector.memset(ones_sb[:], 1.0)
            iota_rep = cpool.tile([P, b_max * P], f32)
            nc.gpsimd.iota(
                iota_rep[:],
                pattern=[[0, b_max], [1, P]],
                base=0,
                channel_multiplier=0,
                allow_small_or_imprecise_dtypes=True,
            )

            for t in range(TILES_PER_CORE):
                c0 = t * b_max
                # gather msgs for this tile's b_max blocks (HW indirect DMA
                # honors ONE index per partition per instruction)
                msg = msgpool.tile([P, b_max * P], gdt, tag="msg")
                for b in range(b_max):
                    nc.gpsimd.indirect_dma_start(
                        out=msg[:, b * P : (b + 1) * P],
                        out_offset=None,
                        in_=nh_full[:, :],
                        in_offset=bass.IndirectOffsetOnAxis(
                            ap=srcidx_sb[:, c0 + b : c0 + b + 1], axis=0
                        ),
                    )
                # all one-hots for the tile in one batched compare
                oh = ohpool.tile([P, b_max * P], gdt, tag="oh")
                nc.vector.tensor_tensor(
                    out=oh[:],
                    in0=iota_rep[:],
                    in1=dstsel_sb[:, c0 : c0 + b_max].to_broadcast([P, b_max, P]),
                    op=mybir.AluOpType.is_equal,
                )
                # segment sum via one-hot matmuls
                agg_ps = ps_agg.tile([P, P], f32, space="PSUM", tag="agg")
                for b in range(b_max):
                    nc.tensor.matmul(
                        out=agg_ps[:],
                        lhsT=msg[:, b * P : (b + 1) * P],
                        rhs=oh[:, b * P : (b + 1) * P],
                        start=(b == 0),
                        stop=(b == b_max - 1),
                    )
                # mean: scale by 1/count (replicated per-node table)
                agg_sb = wpool.tile([P, P], f32, tag="agg_sb")
                nc.vector.tensor_tensor(
                    out=agg_sb[:],
                    in0=agg_ps[:],
                    in1=invcr_sb[:, t * P : (t + 1) * P],
                    op=mybir.AluOpType.mult,
                )

                # MLP layer 1 (transposed layout: [hid, node])
                h1_sb = wpool.tile([P, 2 * P], f32, tag="h1")
                for mc in range(2):
                    h1_ps = ps_h1.tile([P, P], f32, space="PSUM", tag="h1ps")
                    nc.tensor.matmul(
                        out=h1_ps[:],
                        lhsT=W1_sb0[:, mc * P : (mc + 1) * P],
                        rhs=nhT_sb[:, t * P : (t + 1) * P],
                        start=True,
                        stop=False,
                    )
                    nc.tensor.matmul(
                        out=h1_ps[:],
                        lhsT=W1_sb1[:, mc * P : (mc + 1) * P],
                        rhs=agg_sb[:],
                        start=False,
                        stop=True,
                    )
                    nc.scalar.activation(
                        out=h1_sb[:, mc * P : (mc + 1) * P],
                        in_=h1_ps[:],
                        func=mybir.ActivationFunctionType.Relu,
                        bias=b1_sb[:, mc : mc + 1],
                    )

                # MLP layer 2, flipped: psum [node, out_feat]
                h2_ps = ps_h2.tile([P, D_OUT], f32, space="PSUM", tag="h2ps")
                nc.tensor.matmul(
                    out=h2_ps[:], lhsT=ones_sb[:], rhs=b2_sb[:], start=True, stop=False
                )
                nc.tensor.matmul(
                    out=h2_ps[:],
                    lhsT=h1_sb[:, 0:P],
                    rhs=W2_sb0[:],
                    start=False,
                    stop=False,
                )
                nc.tensor.matmul(
                    out=h2_ps[:],
                    lhsT=h1_sb[:, P : 2 * P],
                    rhs=W2_sb1[:],
                    start=False,
                    stop=True,
                )
                out_sb = opool.tile([P, D_OUT], f32, tag="out")
                nc.scalar.activation(
                    out=out_sb[:],
                    in_=h2_ps[:],
                    func=mybir.ActivationFunctionType.Copy,
                )
                nc.sync.dma_start(
                    out=nh_out[t * P : (t + 1) * P, :], in_=out_sb[:]
                )
    return nc


def kernel(nh, eh, W1, b1, W2, b2, edge_index):
    from concourse import bass_utils

    nh = np.asarray(nh, dtype=np.float32)
    b_max, ncols, per_core = _host_prep(nh, edge_index)

    gather_np = {"float32": np.float32, "float16": np.float16}[GATHER_DT]
    nh_gather = np.ascontiguousarray(nh.astype(gather_np))

    nc = _build_bass(b_max, ncols, GATHER_DT)
    _orig_to_json = nc.to_json_bytes
    nc.to_json_bytes = lambda: _split_waits(_orig_to_json())

    W1_a = np.ascontiguousarray(np.asarray(W1, dtype=np.float32))
    W2_a = np.ascontiguousarray(np.asarray(W2, dtype=np.float32))
    b1_a = np.ascontiguousarray(np.asarray(b1, dtype=np.float32).reshape(D_HID, 1))
    b2_a = np.ascontiguousarray(np.asarray(b2, dtype=np.float32).reshape(1, D_OUT))

    in_maps = []
    for k in range(N_CORES):
        pc = per_core[k]
        in_maps.append(
            {
                "nh_full": nh_gather,
                "nhT": pc["nhT"],
                "srcidx": pc["srcidx"],
                "dstsel": pc["dstsel"],
                "invcr": pc["invcr"],
                "W1": W1_a,
                "b1": b1_a,
                "W2": W2_a,
                "b2": b2_a,
            }
        )

    import os
    import time

    res = bass_utils.run_bass_kernel_spmd(nc, in_maps, core_ids=list(range(N_CORES)))
    n_timing = int(os.environ.get("KERNEL_TIME_RUNS", "0"))
    if n_timing:
        times = []
        for _ in range(n_timing):
            t0 = time.perf_counter()
            bass_utils.run_bass_kernel_spmd(nc, in_maps, core_ids=list(range(N_CORES)))
            times.append(time.perf_counter() - t0)
        print(f"HW exec time: {int(min(times) * 1e9)} ns (wall, incl transfer)")
    n_h = np.concatenate([r["nh_out"] for r in res.results], axis=0)[:N_NODES]
    return (n_h, np.asarray(eh))
